# revision 2
# baseline (speedup 1.0000x reference)
"""Trainium2 Bass kernel for nn_LogBessel: out = log(I_31(kappa) + 1e-10).

Math: instead of the reference's 128-term log-space power series, use the
exact identity

    ln I_nu(x) = W + nu*ln(x) - nu*ln(nu + W) - 0.25*ln(W^2)
                 - 0.5*ln(2*pi) + R(t),
    W = sqrt(nu^2 + x^2),  t = nu / W,  nu = 31

where R(t) = ln(sum_k u_k(t)/nu^k) is the (tiny, |R| < 3e-3) residual of the
uniform asymptotic expansion. R is fitted offline as a degree-4 polynomial in
t over t in [0.5269, 1] (max abs error 3.9e-8, far below fp32 noise).

All transcendentals (ln/exp/square, one ACT table set) run on the scalar
engine; W and t come from exp(+-0.5*ln(x^2+961)), avoiding the low-precision
hardware sqrt. The vector engine does 1 tensor_scalar + 7 fused
scalar_tensor_tensor ops (Horner + assembly). Final exp/ln reproduces the
reference's exp(log_iv) + eps -> log structure so the small-x branch
(output == log(1e-10)) matches exactly.

Sharding: trivially data-parallel; rows 4096 split into 8 blocks of 512 rows,
one per NeuronCore (same SPMD program, different data).
"""

import numpy as np

from concourse import bacc, mybir, tile
from concourse import bass_utils

F32 = mybir.dt.float32
AF = mybir.ActivationFunctionType
OP = mybir.AluOpType

N_CORES = 8
ROWS, COLS = 4096, 4096
SH_ROWS = ROWS // N_CORES          # 512 rows per core
P = 128                            # SBUF partitions
FD = 2048                          # free-dim chunk size
ROW_BLOCKS = SH_ROWS // P          # 4
COL_BLOCKS = COLS // FD            # 2

# deg-4 fit of R(t) on [0.52694, 1.0], power basis (see module docstring)
A0 = 0.0002118241727494352
A1 = 0.002582156236836899
A2 = 0.00392171351050473
A3 = -0.01156101279183025
A4 = 0.0021572046418213957
LN31 = 3.4339872044851463          # ln(31)
K0 = -0.9189385332046728           # -0.5*ln(2*pi)
C0 = A0 + K0                       # folded constant term
EPS = 1e-10

_nc_cache = None


def _build():
    nc = bacc.Bacc("TRN2", target_bir_lowering=False, debug=False)
    x = nc.dram_tensor("x", [SH_ROWS, COLS], F32, kind="ExternalInput").ap()
    y = nc.dram_tensor("y", [SH_ROWS, COLS], F32, kind="ExternalOutput").ap()

    # activation() requires float biases to exist as [128,1] const SBUF
    # tensors; register ours the same way Bass.__init__ registers 0.0/1.0.
    for val in (961.0, LN31, 31.0, EPS):
        t = nc.alloc_sbuf_tensor(f"const-f32-{val}", [128, 1], F32)
        nc.gpsimd.memset(t.ap(), val)
        nc.const_aps.aps[(F32, val)] = t.ap()
    nc.all_engine_barrier()

    with tile.TileContext(nc) as tc:
        with tc.tile_pool(name="p", bufs=2) as pool:
            for c in range(ROW_BLOCKS):
                for d in range(COL_BLOCKS):
                    rs = slice(c * P, (c + 1) * P)
                    cs = slice(d * FD, (d + 1) * FD)

                    tx = pool.tile([P, FD], F32, tag="x")
                    nc.sync.dma_start(tx[:], x[rs, cs])

                    tx2 = pool.tile([P, FD], F32, tag="x2")
                    nc.scalar.activation(tx2[:], tx[:], AF.Square)
                    tL = pool.tile([P, FD], F32, tag="L")
                    nc.scalar.activation(tL[:], tx[:], AF.Ln)
                    ty = pool.tile([P, FD], F32, tag="y")
                    nc.scalar.activation(ty[:], tx2[:], AF.Ln, bias=961.0)
                    tW = pool.tile([P, FD], F32, tag="W")
                    nc.scalar.activation(tW[:], ty[:], AF.Exp, scale=0.5)
                    tt = pool.tile([P, FD], F32, tag="t")
                    nc.scalar.activation(tt[:], ty[:], AF.Exp,
                                         scale=-0.5, bias=LN31)
                    tq = pool.tile([P, FD], F32, tag="q")
                    nc.scalar.activation(tq[:], tW[:], AF.Ln, bias=31.0)

                    # Horner for R(t): G = a4*t; G = (G + a_j)*t
                    tG = pool.tile([P, FD], F32, tag="G")
                    nc.vector.tensor_scalar_mul(tG[:], tt[:], A4)
                    nc.vector.scalar_tensor_tensor(
                        tG[:], tG[:], A3, tt[:], op0=OP.add, op1=OP.mult)
                    nc.vector.scalar_tensor_tensor(
                        tG[:], tG[:], A2, tt[:], op0=OP.add, op1=OP.mult)
                    nc.vector.scalar_tensor_tensor(
                        tG[:], tG[:], A1, tt[:], op0=OP.add, op1=OP.mult)

                    # assembly: g = W - 31*ln(31+W) + 31*ln(x) - 0.25*y
                    #               + G + (a0 - 0.5*ln(2pi))
                    ts_ = pool.tile([P, FD], F32, tag="s")
                    nc.vector.scalar_tensor_tensor(
                        ts_[:], tq[:], -31.0, tW[:], op0=OP.mult, op1=OP.add)
                    nc.vector.scalar_tensor_tensor(
                        ts_[:], tL[:], 31.0, ts_[:], op0=OP.mult, op1=OP.add)
                    nc.vector.scalar_tensor_tensor(
                        ts_[:], ty[:], -0.25, ts_[:], op0=OP.mult, op1=OP.add)
                    nc.vector.scalar_tensor_tensor(
                        ts_[:], tG[:], C0, ts_[:], op0=OP.add, op1=OP.add)

                    # out = ln(exp(g) + eps)  (same structure as reference)
                    to = pool.tile([P, FD], F32, tag="o")
                    nc.scalar.activation(to[:], ts_[:], AF.Exp)
                    nc.scalar.activation(to[:], to[:], AF.Ln, bias=EPS)

                    nc.sync.dma_start(y[rs, cs], to[:])

    nc.compile()
    return nc


def _get_nc():
    global _nc_cache
    if _nc_cache is None:
        _nc_cache = _build()
    return _nc_cache


def kernel(kappa: np.ndarray) -> np.ndarray:
    kappa = np.ascontiguousarray(np.asarray(kappa, dtype=np.float32))
    assert kappa.shape == (ROWS, COLS)
    nc = _get_nc()
    in_maps = [
        {"x": kappa[i * SH_ROWS:(i + 1) * SH_ROWS]} for i in range(N_CORES)
    ]
    res = bass_utils.run_bass_kernel_spmd(
        nc, in_maps, core_ids=list(range(N_CORES)))
    out = np.concatenate([res.results[i]["y"] for i in range(N_CORES)], axis=0)
    return out.astype(np.float32)
